# revision 1
# baseline (speedup 1.0000x reference)
"""Trainium2 Bass kernel for nn_Conv_agg (edge-parallel GNN message passing).

Math (see reference):
    out[n] = sum_k ( sum_{e: src(e)=n} X[e,k] * h[tgt(e)] ) @ W[k] + bias

Structure exploited (asserted at runtime, guaranteed by setup_inputs):
  - src(e) = e // DEG exactly (each node emits DEG=16 consecutive edges)
  - edges/nodes of graph g are contiguous and tgt(e) stays inside graph g's
    100-node window -> the whole problem is block-diagonal over graphs.

Dense per-graph formulation (no gather at all):
    M_k[s,t] = sum_{e in seg(s), tgt_e=t} X[e,k]      (100x100 per graph, per k)
    out_g    = sum_k M_k @ (h_g @ W_k) + bias

Per-core device pipeline (125 graphs/core, all bf16 on the PE):
  1. DVE:  O[e,t] = (tgt_e == t) one-hot via is_equal vs iota const
  2. Pool: Xall[e,(s,k)] = X[e,k] * blockdiag_mask (8 sources per 128-edge blk)
  3. PE:   M^T[t,(s,k)] block = O_b^T @ Xall_b, 13 blocks of 128 edges
  4. Act:  copy M^T PSUM -> SBUF bf16, de-interleaving k
  5. PE:   hW[t,(k,o)] = h_g^T.T @ [W0|W1]   (h^T preloaded, host-transposed)
  6. Pool: copy hW PSUM -> SBUF bf16
  7. PE:   out[s,o] = sum_k M_k^T.T @ hW_k   (PSUM accumulate over k)
  8. DVE adds bias (f32), DMA out rows.
"""

import numpy as np

B, NPG, DEG, K, CIN, COUT = 1000, 100, 16, 2, 128, 128
E = B * NPG * DEG            # 1,600,000 edges
NT = B * NPG                 # 100,000 nodes
NCORES = 8
G_C = B // NCORES            # 125 graphs / core
NT_C = NT // NCORES          # 12,500 nodes / core
E_C = E // NCORES            # 200,000 edges / core
EPG = NPG * DEG              # 1600 edges / graph
NB = -(-EPG // 128)          # 13 blocks of 128 edges (last half-padded)
EPG_P = NB * 128             # 1664
SPB = 128 // DEG             # 8 sources per 128-edge block
S_P = NB * SPB               # 104 source slots (100 real + 4 pad)

_module_cache = {}


def _patch_tile_drain():
    """This walrus build allows a single sync-wait per instruction; Tile's
    kernel-tail drain aggregates one wait per outstanding sem onto one
    InstDrain. Hoist extras onto dedicated sync nops (sequential on SP)."""
    import concourse.mybir as mybir
    from concourse.tile import TileContext
    from concourse.vector_clock import ScopedClock

    if getattr(TileContext, "_drain_patched", False):
        return

    def _drain_and_barrier(self, tick_clock, wait_clock):
        probe = self.nc.sync.nop(nofuse=True)
        wait_clock.add_sem_waits(probe.ins, ScopedClock({None: tick_clock.global_clock}))
        si = probe.ins.sync_info
        waits = list(si.on_wait) if si is not None and si.on_wait else []
        if si is not None and len(waits) > 1:
            si.on_wait = waits[:1]
            for w in waits[1:]:
                n = self.nc.sync.nop(nofuse=True)
                n.ins.sync_info = mybir.SyncInfo(on_wait=[w], on_update=[])
        self.nc.sync.drain()
        self.nc.all_engine_barrier()
        assert self.sems is not None
        popped = self.nc._tile_sem_poison_stack.pop()
        assert popped is self._sem_poison
        self.nc.clear_and_free_semaphores(list(self.sems.allocated().values()))
        self.nc.all_engine_barrier()

    TileContext._drain_and_barrier = _drain_and_barrier
    TileContext._drain_patched = True


def _build_module(with_bias):
    import concourse.bacc as bacc
    import concourse.mybir as mybir
    from concourse.tile import TileContext

    _patch_tile_drain()
    f32 = mybir.dt.float32
    bf16 = mybir.dt.bfloat16
    fp8 = mybir.dt.float8e4

    nc = bacc.Bacc("TRN2", target_bir_lowering=False)
    ht_t = nc.dram_tensor("ht", [CIN, NT_C], bf16, kind="ExternalInput")
    xa_t = nc.dram_tensor("xa", [128, G_C, NB, SPB * K], bf16,
                          kind="ExternalInput")
    tg_t = nc.dram_tensor("tg", [128, G_C, NB], bf16, kind="ExternalInput")
    w_t = nc.dram_tensor("w", [CIN, K, COUT], bf16, kind="ExternalInput")
    iota_t = nc.dram_tensor("iota", [128, NPG], bf16, kind="ExternalInput")
    if with_bias:
        ones_t = nc.dram_tensor("ones", [1, S_P], bf16, kind="ExternalInput")
        bias_t = nc.dram_tensor("bias", [1, COUT], bf16, kind="ExternalInput")
    out_t = nc.dram_tensor("out", [NT_C, COUT], f32, kind="ExternalOutput")

    with TileContext(nc) as tc:
        with (
            tc.tile_pool(name="consts", bufs=1) as cpool,
            tc.tile_pool(name="op", bufs=4) as opool,
            tc.tile_pool(name="mtp", bufs=3) as mtpool,
            tc.tile_pool(name="hwp", bufs=3) as hwpool,
            tc.tile_pool(name="outp", bufs=3) as outpool,
            tc.tile_pool(name="psM", bufs=3, space="PSUM") as psM,
            tc.tile_pool(name="psH", bufs=2, space="PSUM") as psH,
            tc.tile_pool(name="psO", bufs=3, space="PSUM") as psO,
        ):
            ht_sb = cpool.tile([CIN, NT_C], bf16)
            nc.sync.dma_start(ht_sb[:, :], ht_t[:, :])
            xa_sb = cpool.tile([128, G_C, NB, SPB * K], bf16)
            nc.sync.dma_start(xa_sb[:, :, :, :], xa_t[:, :, :, :])
            tg_sb = cpool.tile([128, G_C, NB], bf16)
            nc.sync.dma_start(tg_sb[:, :, :], tg_t[:, :, :])
            w_sb = cpool.tile([CIN, K, COUT], bf16)
            nc.sync.dma_start(w_sb[:, :, :], w_t[:, :, :])
            iota_sb = cpool.tile([128, NPG], bf16)
            nc.sync.dma_start(iota_sb[:, :], iota_t[:, :])
            if with_bias:
                ones_sb = cpool.tile([1, S_P], bf16)
                nc.sync.dma_start(ones_sb[:, :], ones_t[:, :])
                bias_sb = cpool.tile([1, COUT], bf16)
                nc.sync.dma_start(bias_sb[:, :], bias_t[:, :])

            # software pipeline: head(g) feeds PE; tail(g-1) overlaps with
            # head(g+1) so the PE never stalls on the Act copies.
            mt_tiles = {}
            hw_tiles = {}

            def head(g):
                # 1. one-hot O[e_part, blk, t] = (tgt == t); [b, t] layout
                # keeps the per-block lhsT slices contiguous for ldweights
                o_sb = opool.tile([128, NB, NPG], bf16)
                nc.vector.tensor_tensor(
                    o_sb[:, :, :],
                    tg_sb[:, g, :].unsqueeze(2).broadcast_to([128, NB, NPG]),
                    iota_sb[:, :].unsqueeze(1).broadcast_to([128, NB, NPG]),
                    op=mybir.AluOpType.is_equal,
                )
                # 3. M^T[t, blk, (s,k)] = O_b^T @ Xall_b per 128-edge block
                # (Xall precomputed host-side: X * blockdiag mask)
                psM_tl = psM.tile([NPG, NB, SPB * K], f32)
                for b in range(NB):
                    nc.tensor.matmul(
                        psM_tl[:, b, :],
                        o_sb[:, b, :],           # lhsT [128e, 100t] contiguous
                        xa_sb[:, g, b, :],       # rhs  [128e, 16]
                        start=True, stop=True,
                    )
                # 5. hW[t, (k,o)] = h_g @ [W0|W1]
                psH_tl = psH.tile([NPG, K, COUT], f32)
                nc.tensor.matmul(
                    psH_tl[:, :, :].rearrange("p k o -> p (k o)"),
                    ht_sb[:, g * NPG:(g + 1) * NPG],   # lhsT [128c, 100t]
                    w_sb[:, :, :].rearrange("c k o -> c (k o)"),
                    start=True, stop=True,
                )
                # 4/6. PSUM -> SBUF bf16 on Act, contiguous copies; the
                # d-stage eats the k-interleave as a cheap stride-2 ldweights
                mt_sb = mtpool.tile([NPG, NB, SPB, K], bf16)
                nc.scalar.copy(mt_sb[:, :, :, :],
                               psM_tl[:, :, :].rearrange("p b (s k) -> p b s k", k=K))
                hw_sb = hwpool.tile([NPG, K, COUT], bf16)
                nc.scalar.copy(hw_sb[:, :, :], psH_tl[:, :, :])
                mt_tiles[g] = mt_sb
                hw_tiles[g] = hw_sb

            def tail(g):
                mt_sb = mt_tiles.pop(g)
                hw_sb = hw_tiles.pop(g)
                # 7. out[s, o] = sum_k M_k^T.T @ hW_k  (+ ones x bias)
                psO_tl = psO.tile([S_P, COUT], f32)
                nmm = K + (1 if with_bias else 0)
                for k in range(K):
                    nc.tensor.matmul(
                        psO_tl[:, :],
                        mt_sb[:, :, :, k].rearrange("p b s -> p (b s)"),
                        hw_sb[:, k, :],
                        start=(k == 0), stop=(k == nmm - 1),
                    )
                if with_bias:
                    nc.tensor.matmul(psO_tl[:, :], ones_sb[:, :], bias_sb[:, :],
                                     start=False, stop=True)
                # 8. PSUM -> SBUF on Act (mid-chain engine; keeps DVE free
                # to run O-builds ahead), then store
                o_out = outpool.tile([NPG, COUT], f32)
                nc.scalar.copy(o_out[:, :], psO_tl[:NPG, :])
                nc.sync.dma_start(out_t[g * NPG:(g + 1) * NPG, :], o_out[:, :])

            for g in range(G_C):
                head(g)
                if g >= 1:
                    tail(g - 1)
            tail(G_C - 1)
    nc.compile()
    return nc


def _get_module(with_bias):
    key = ("nc", with_bias)
    if key not in _module_cache:
        _module_cache[key] = _build_module(with_bias)
    return _module_cache[key]


def _prep_inputs(h, X, tgt, weight, bias):
    """Host-side sharding/layout (no arithmetic on data values)."""
    import ml_dtypes
    bf16 = ml_dtypes.bfloat16

    g_all = np.arange(E, dtype=np.int64) // EPG      # graph id per edge
    tloc = tgt - g_all * NPG                         # within-graph target
    assert tloc.min() >= 0 and tloc.max() < NPG, "tgt escapes graph block"

    tlp = np.zeros((NCORES, G_C, EPG_P), np.float32)
    tlp[:, :, :EPG] = tloc.reshape(NCORES, G_C, EPG)
    Xp = np.zeros((NCORES, G_C, EPG_P, K), np.float32)
    Xp[:, :, :EPG] = X.reshape(NCORES, G_C, EPG, K)

    # e = 128*b + p  ->  [core, p, g, b(, k)]
    tg_arr = np.ascontiguousarray(
        tlp.reshape(NCORES, G_C, NB, 128).transpose(0, 3, 1, 2)).astype(bf16)
    xr_arr = np.ascontiguousarray(
        Xp.reshape(NCORES, G_C, NB, 128, K).transpose(0, 3, 1, 2, 4)).astype(bf16)
    # Xall[c, p, g, b, (s k)] = X * (p//16 == s) block-diagonal expansion
    mask8 = (np.arange(128)[:, None] // DEG
             == np.arange(SPB)[None, :]).astype(np.float32)   # [128, 8]
    xa_arr = (xr_arr.astype(np.float32)[:, :, :, :, None, :]
              * mask8[None, :, None, None, :, None]).astype(bf16)
    xa_arr = np.ascontiguousarray(
        xa_arr.reshape(NCORES, 128, G_C, NB, SPB * K))

    ht = np.ascontiguousarray(
        h.astype(bf16).reshape(NCORES, NT_C, CIN).transpose(0, 2, 1))

    iota = np.ascontiguousarray(np.broadcast_to(
        np.arange(NPG, dtype=np.float32), (128, NPG))).astype(bf16)
    w2 = np.ascontiguousarray(weight.transpose(1, 0, 2)).astype(bf16)
    ones = np.ones((1, S_P), bf16)
    bias_row = bias.reshape(1, COUT).astype(bf16)
    return ht, xa_arr, tg_arr, w2, iota, ones, bias_row


def kernel(h, X, edge_index, node_index, batch_node, batch_edge, num_node,
           weight, bias):
    from concourse.bass_utils import run_bass_kernel_spmd

    h = np.asarray(h, np.float32)
    X = np.asarray(X, np.float32)
    edge_index = np.asarray(edge_index)
    weight = np.asarray(weight, np.float32)
    bias = np.asarray(bias, np.float32)

    src = np.asarray(edge_index[1])
    tgt = np.asarray(edge_index[2])
    # structural contract from setup_inputs (see module docstring)
    assert src.shape == (E,) and h.shape == (NT, CIN) and X.shape == (E, K)
    assert np.array_equal(src, np.arange(E, dtype=src.dtype) // DEG), \
        "edges not sorted as src=e//DEG"

    ht, xa_arr, tg_arr, w2, iota, ones, bias_row = _prep_inputs(
        h, X, tgt, weight, bias)

    with_bias = bool(np.any(bias))
    nc = _get_module(with_bias)
    in_maps = []
    for c in range(NCORES):
        m = {
            "ht": ht[c],
            "xa": xa_arr[c],
            "tg": tg_arr[c],
            "w": w2,
            "iota": iota,
        }
        if with_bias:
            m["ones"] = ones
            m["bias"] = bias_row
        in_maps.append(m)
    res = run_bass_kernel_spmd(nc, in_maps, core_ids=list(range(NCORES)))
    out = np.concatenate([r["out"] for r in res.results], axis=0)
    return out



# revision 3
# speedup vs baseline: 1.4675x; 1.4675x over previous
"""Trainium2 Bass kernel for nn_Conv_agg (edge-parallel GNN message passing).

Math (see reference):
    out[n] = sum_k ( sum_{e: src(e)=n} X[e,k] * h[tgt(e)] ) @ W[k] + bias

Structure exploited (guaranteed by setup_inputs):
  - src(e) = e // DEG exactly (each node emits DEG=16 consecutive edges)
  - edges/nodes of graph g are contiguous; tgt(e) stays inside graph g's
    100-node window -> block-diagonal over graphs.

Per-graph dense formulation:
    M^T[t, k, s] = sum_{e: src=s, tgt=t} X[e,k]          (the edge aggregate)
    P'[c, k, s]  = sum_t h_g[t, c] * M^T[t, k, s]        (A' matmul)
    out^T[o, s]  = sum_k sum_c W_k[c, o] * P'[c, k, s] + bias[o]   (B' matmuls)

M^T is built almost entirely by HOST LAYOUT (placing X values into a dense
[112, 2, 104] tile - zero host arithmetic).  Colliding (s,t) edge pairs
cannot share a slot; the 12 spare partition rows (100..111) absorb the
most-frequent duplicate targets per graph (their h rows are host-copied
into h_aug rows 100..111), and the residual duplicates (<=128 per graph,
seed-0 max 109) go through one small on-device one-hot matmul per graph
whose PSUM result is accumulated into P' by a second A' matmul.

Per-core device pipeline (125 graphs/core):
  DVE:  O_t[e,t] = (tg_rem[e]==t), S[e,s] = (sg_rem[e]==s), XS = S*X_k
        (tensor_scalar is_equal/mult with per-partition [P,1] scalars -> 4x)
  PE:   psumR = O_t^T @ XS                (residual-duplicate M^T)
  Act:  Mrem = copy(psumR) bf16
  PE:   psumP = h_aug_g^T.T @ [M^T | Mrem]  (2 accumulating matmuls, N=208)
  DVE:  P' = copy(psumP) bf16
  PE:   psumO = sum_k W_k.T @ P'_k        (2 accumulating matmuls, N=100)
  Act:  out^T = Identity(psumO + bias), DMA out (host re-transposes)
"""

import numpy as np

B, NPG, DEG, K, CIN, COUT = 1000, 100, 16, 2, 128, 128
E = B * NPG * DEG            # 1,600,000 edges
NT = B * NPG                 # 100,000 nodes
NCORES = 8
G_C = B // NCORES            # 125 graphs / core
EXTRA = 12                   # spare partition rows for duplicate absorption
TP = NPG + EXTRA             # 112 partition rows of M^T / h_aug
SP = 104                     # padded source-slot count (100 used)
REMC = 128                   # residual-duplicate capacity per graph per block

_module_cache = {}


def _patch_tile_drain():
    """This walrus build allows a single sync-wait per instruction; Tile's
    kernel-tail drain aggregates one wait per outstanding sem onto one
    InstDrain. Hoist extras onto dedicated sync nops (sequential on SP)."""
    import concourse.mybir as mybir
    from concourse.tile import TileContext
    from concourse.vector_clock import ScopedClock

    if getattr(TileContext, "_drain_patched", False):
        return

    def _drain_and_barrier(self, tick_clock, wait_clock):
        probe = self.nc.sync.nop(nofuse=True)
        wait_clock.add_sem_waits(probe.ins, ScopedClock({None: tick_clock.global_clock}))
        si = probe.ins.sync_info
        waits = list(si.on_wait) if si is not None and si.on_wait else []
        if si is not None and len(waits) > 1:
            si.on_wait = waits[:1]
            for w in waits[1:]:
                n = self.nc.sync.nop(nofuse=True)
                n.ins.sync_info = mybir.SyncInfo(on_wait=[w], on_update=[])
        self.nc.sync.drain()
        self.nc.all_engine_barrier()
        assert self.sems is not None
        popped = self.nc._tile_sem_poison_stack.pop()
        assert popped is self._sem_poison
        self.nc.clear_and_free_semaphores(list(self.sems.allocated().values()))
        self.nc.all_engine_barrier()

    TileContext._drain_and_barrier = _drain_and_barrier
    TileContext._drain_patched = True


def _build_module(nb_rem):
    import concourse.bacc as bacc
    import concourse.mybir as mybir
    from concourse.tile import TileContext

    _patch_tile_drain()
    f32 = mybir.dt.float32
    bf16 = mybir.dt.bfloat16
    AF = mybir.ActivationFunctionType
    NR = nb_rem * REMC

    nc = bacc.Bacc("TRN2", target_bir_lowering=False)
    h_t = nc.dram_tensor("h", [TP, G_C, CIN], bf16, kind="ExternalInput")
    m_t = nc.dram_tensor("m", [TP, G_C, K, SP], bf16, kind="ExternalInput")
    tg_t = nc.dram_tensor("tg", [NR, G_C], f32, kind="ExternalInput")
    sg_t = nc.dram_tensor("sg", [NR, G_C], f32, kind="ExternalInput")
    xr_t = nc.dram_tensor("xr", [NR, G_C, K], f32, kind="ExternalInput")
    w_t = nc.dram_tensor("w", [CIN, K, COUT], bf16, kind="ExternalInput")
    bias_t = nc.dram_tensor("bias", [COUT, 1], f32, kind="ExternalInput")
    iot_t = nc.dram_tensor("iot", [REMC, 128], bf16, kind="ExternalInput")
    ios_t = nc.dram_tensor("ios", [REMC, SP], bf16, kind="ExternalInput")
    out_t = nc.dram_tensor("out", [COUT, G_C, NPG], f32, kind="ExternalOutput")

    NCHUNK = 5                        # input DMA split so graph 0 starts early
    CG = G_C // NCHUNK                # 25 graphs per chunk

    with TileContext(nc) as tc:
        with (
            tc.tile_pool(name="consts", bufs=1) as cpool,
            tc.tile_pool(name="onehot", bufs=3) as opool,
            tc.tile_pool(name="mrem", bufs=3) as mpool,
            tc.tile_pool(name="pp", bufs=3) as ppool,
            tc.tile_pool(name="outp", bufs=3) as outpool,
            tc.tile_pool(name="psR", bufs=2, space="PSUM") as psR,
            tc.tile_pool(name="psP", bufs=2, space="PSUM") as psP,
            tc.tile_pool(name="psO", bufs=3, space="PSUM") as psO,
        ):
            h_sb = cpool.tile([TP, G_C, CIN], bf16)
            m_sb = cpool.tile([TP, G_C, K, SP], bf16)
            tg_sb = cpool.tile([NR, G_C], f32)
            sg_sb = cpool.tile([NR, G_C], f32)
            xr_sb = cpool.tile([NR, G_C, K], f32)
            for c0 in range(0, G_C, CG):
                c1 = c0 + CG
                nc.sync.dma_start(h_sb[:, c0:c1], h_t[:, c0:c1])
                nc.sync.dma_start(m_sb[:, c0:c1], m_t[:, c0:c1])
                nc.sync.dma_start(tg_sb[:, c0:c1], tg_t[:, c0:c1])
                nc.sync.dma_start(sg_sb[:, c0:c1], sg_t[:, c0:c1])
                nc.sync.dma_start(xr_sb[:, c0:c1], xr_t[:, c0:c1])
            w_sb = cpool.tile([CIN, K, COUT], bf16)
            nc.sync.dma_start(w_sb[:, :, :], w_t[:, :, :])
            bias_sb = cpool.tile([COUT, 1], f32)
            nc.sync.dma_start(bias_sb[:, :], bias_t[:, :])
            iot_sb = cpool.tile([REMC, 128], bf16)
            nc.sync.dma_start(iot_sb[:, :], iot_t[:, :])
            ios_sb = cpool.tile([REMC, SP], bf16)
            nc.sync.dma_start(ios_sb[:, :], ios_t[:, :])

            mrem_tiles = {}
            pp_tiles = {}

            def stage_build(g):
                # residual-duplicate M^T via one-hot matmul per 128-edge block
                mrem_sb = mpool.tile([TP, nb_rem, K, SP], bf16)
                for rb in range(nb_rem):
                    e0 = rb * REMC
                    e1 = e0 + REMC
                    ot = opool.tile([REMC, 128], bf16)
                    nc.vector.tensor_scalar(
                        ot[:, :], iot_sb[:, :], tg_sb[e0:e1, g:g + 1], None,
                        op0=mybir.AluOpType.is_equal)
                    s_oh = opool.tile([REMC, SP], bf16)
                    nc.vector.tensor_scalar(
                        s_oh[:, :], ios_sb[:, :], sg_sb[e0:e1, g:g + 1], None,
                        op0=mybir.AluOpType.is_equal)
                    xs = opool.tile([REMC, K, SP], bf16)
                    for k in range(K):
                        nc.vector.tensor_scalar(
                            xs[:, k, :], s_oh[:, :], xr_sb[e0:e1, g, k:k + 1],
                            None, op0=mybir.AluOpType.mult)
                    psumR = psR.tile([128, K, SP], f32)
                    nc.tensor.matmul(
                        psumR[:, :, :].rearrange("p k s -> p (k s)"),
                        ot[:, :], xs[:, :, :].rearrange("p k s -> p (k s)"),
                        start=True, stop=True)
                    nc.scalar.copy(mrem_sb[:, rb, :, :], psumR[:TP, :, :])
                mrem_tiles[g] = mrem_sb

            def stage_a(g):
                mrem_sb = mrem_tiles.pop(g)
                psumP = psP.tile([CIN, K, SP], f32)
                nc.tensor.matmul(
                    psumP[:, :, :].rearrange("p k s -> p (k s)"),
                    h_sb[:, g, :],
                    m_sb[:, g, :, :].rearrange("p k s -> p (k s)"),
                    start=True, stop=False)
                nc.tensor.matmul(
                    psumP[:, :, :].rearrange("p k s -> p (k s)"),
                    h_sb[:, g, :],
                    mrem_sb[:, :, :, :].rearrange("p r k s -> p (r k s)"),
                    start=False, stop=True)
                pp_sb = ppool.tile([CIN, K, SP], bf16)
                nc.vector.tensor_copy(pp_sb[:, :, :], psumP[:, :, :])
                pp_tiles[g] = pp_sb

            def stage_b(g):
                pp_sb = pp_tiles.pop(g)
                psumO = psO.tile([COUT, NPG], f32)
                for k in range(K):
                    nc.tensor.matmul(
                        psumO[:, :], w_sb[:, k, :], pp_sb[:, k, :NPG],
                        start=(k == 0), stop=(k == K - 1))
                o_sb = outpool.tile([COUT, NPG], f32)
                nc.scalar.activation(o_sb[:, :], psumO[:, :], AF.Identity,
                                     bias=bias_sb[:, :], scale=1.0)
                nc.sync.dma_start(out_t[:, g, :], o_sb[:, :])

            # A' matmul for rem of graph g uses Mrem written by Act; skew the
            # stages so PE never waits: build(g) | a(g-1) | b(g-2)
            for g in range(G_C):
                stage_build(g)
                if g >= 1:
                    stage_a(g - 1)
                if g >= 2:
                    stage_b(g - 2)
            stage_a(G_C - 1)
            stage_b(G_C - 2)
            stage_b(G_C - 1)
    nc.compile()
    return nc


def _get_module(nb_rem):
    key = ("nc", nb_rem)
    if key not in _module_cache:
        _module_cache[key] = _build_module(nb_rem)
    return _module_cache[key]


def _prep_inputs(h, X, tgt, weight, bias):
    """Host-side sharding/layout (data PLACEMENT only - no arithmetic)."""
    import ml_dtypes
    bf16 = ml_dtypes.bfloat16

    g_all = (np.arange(E, dtype=np.int64) // (NPG * DEG))
    tloc = (tgt - g_all * NPG).astype(np.int64)
    sloc = (np.arange(E, dtype=np.int64) // DEG - g_all * NPG)
    assert tloc.min() >= 0 and tloc.max() < NPG, "tgt escapes graph block"

    # occurrence index of each edge within its (g, s, t) group
    key = g_all * (NPG * NPG) + sloc * NPG + tloc
    order = np.argsort(key, kind="stable")
    ks = key[order]
    occ_sorted = np.arange(E) - np.searchsorted(ks, ks)
    occ = np.empty(E, np.int64)
    occ[order] = occ_sorted

    first = occ == 0
    M = np.zeros((B, TP, K, SP), np.float32)
    M[g_all[first], tloc[first], 0, sloc[first]] = X[first, 0]
    M[g_all[first], tloc[first], 1, sloc[first]] = X[first, 1]

    # duplicate edges: absorb the most-frequent duplicate targets into the
    # EXTRA spare rows (h row copies), residue goes to the on-device block
    dup = ~first
    cnt = np.zeros((B, NPG), np.int64)
    np.add.at(cnt, (g_all[dup], tloc[dup]), 1)
    top = np.argpartition(-cnt, EXTRA - 1, axis=1)[:, :EXTRA]   # [B, EXTRA]
    row_of_t = np.full((B, NPG), -1, np.int64)
    for r in range(EXTRA - 1, -1, -1):     # reverse so r=0 wins collisions
        row_of_t[np.arange(B), top[:, r]] = r
    r_e = np.where(dup, row_of_t[g_all, tloc], -1)
    mapped = dup & (r_e >= 0)

    # within (g, extra-row, s), only the first mapped edge takes the slot
    mkey = g_all * (EXTRA * NPG) + r_e * NPG + sloc
    mkey = np.where(mapped, mkey, -1)
    mi = np.nonzero(mapped)[0]
    morder = mi[np.argsort(mkey[mi], kind="stable")]
    mks = mkey[morder]
    mocc_sorted = np.arange(len(mi)) - np.searchsorted(mks, mks)
    placed2 = np.zeros(E, bool)
    p2 = morder[mocc_sorted == 0]
    placed2[p2] = True
    M[g_all[p2], NPG + r_e[p2], 0, sloc[p2]] = X[p2, 0]
    M[g_all[p2], NPG + r_e[p2], 1, sloc[p2]] = X[p2, 1]

    rem = dup & ~placed2
    ri = np.nonzero(rem)[0]
    rg = g_all[ri]
    rorder = ri[np.argsort(rg, kind="stable")]
    rgs = g_all[rorder]
    rpos = np.arange(len(ri)) - np.searchsorted(rgs, rgs)
    nrem_max = int(rpos.max()) + 1 if len(ri) else 0
    nb_rem = max(1, -(-nrem_max // REMC))
    NR = nb_rem * REMC
    tg_r = np.zeros((B, NR), np.float32)
    sg_r = np.zeros((B, NR), np.float32)
    x_r = np.zeros((B, NR, K), np.float32)
    tg_r[rgs, rpos] = tloc[rorder]
    sg_r[rgs, rpos] = sloc[rorder]
    x_r[rgs, rpos, 0] = X[rorder, 0]
    x_r[rgs, rpos, 1] = X[rorder, 1]

    # augmented h: rows 100..111 are copies of the absorbed duplicate targets
    h_aug = np.zeros((B, TP, CIN), np.float32)
    h_aug[:, :NPG] = h.reshape(B, NPG, CIN)
    gi = np.repeat(np.arange(B), EXTRA)
    h_aug[:, NPG:] = h.reshape(B, NPG, CIN)[gi, top.ravel()].reshape(B, EXTRA, CIN)

    # per-core layouts (partition dim first)
    m_arr = np.ascontiguousarray(
        M.reshape(NCORES, G_C, TP, K, SP).transpose(0, 2, 1, 3, 4)).astype(bf16)
    h_arr = np.ascontiguousarray(
        h_aug.reshape(NCORES, G_C, TP, CIN).transpose(0, 2, 1, 3)).astype(bf16)
    tg_arr = np.ascontiguousarray(
        tg_r.reshape(NCORES, G_C, NR).transpose(0, 2, 1))
    sg_arr = np.ascontiguousarray(
        sg_r.reshape(NCORES, G_C, NR).transpose(0, 2, 1))
    xr_arr = np.ascontiguousarray(
        x_r.reshape(NCORES, G_C, NR, K).transpose(0, 2, 1, 3))
    w_arr = np.ascontiguousarray(weight.transpose(1, 0, 2)).astype(bf16)
    bias_arr = np.ascontiguousarray(bias.reshape(COUT, 1).astype(np.float32))
    iot = np.ascontiguousarray(np.broadcast_to(
        np.arange(128, dtype=np.float32), (REMC, 128))).astype(bf16)
    ios = np.ascontiguousarray(np.broadcast_to(
        np.arange(SP, dtype=np.float32), (REMC, SP))).astype(bf16)
    return (m_arr, h_arr, tg_arr, sg_arr, xr_arr, w_arr, bias_arr, iot, ios,
            nb_rem)


def kernel(h, X, edge_index, node_index, batch_node, batch_edge, num_node,
           weight, bias):
    from concourse.bass_utils import run_bass_kernel_spmd

    h = np.asarray(h, np.float32)
    X = np.asarray(X, np.float32)
    edge_index = np.asarray(edge_index)
    weight = np.asarray(weight, np.float32)
    bias = np.asarray(bias, np.float32)

    src = np.asarray(edge_index[1])
    tgt = np.asarray(edge_index[2])
    assert src.shape == (E,) and h.shape == (NT, CIN) and X.shape == (E, K)
    assert np.array_equal(src, np.arange(E, dtype=src.dtype) // DEG), \
        "edges not sorted as src=e//DEG"

    (m_arr, h_arr, tg_arr, sg_arr, xr_arr, w_arr, bias_arr, iot, ios,
     nb_rem) = _prep_inputs(h, X, tgt, weight, bias)

    nc = _get_module(nb_rem)
    in_maps = []
    for c in range(NCORES):
        in_maps.append({
            "h": h_arr[c], "m": m_arr[c], "tg": tg_arr[c], "sg": sg_arr[c],
            "xr": xr_arr[c], "w": w_arr, "bias": bias_arr, "iot": iot,
            "ios": ios,
        })
    res = run_bass_kernel_spmd(nc, in_maps, core_ids=list(range(NCORES)))
    # out is [COUT, G_C, NPG] per core -> [NT, COUT]
    outs = [np.asarray(r["out"]).transpose(1, 2, 0).reshape(G_C * NPG, COUT)
            for r in res.results]
    return np.ascontiguousarray(np.concatenate(outs, axis=0))
